# revision 60
# baseline (speedup 1.0000x reference)
import sys
import numpy as np
import ml_dtypes

sys.path.insert(0, "/opt/trn_rl_repo")

import concourse.bass as bass
import concourse.tile as tile
from concourse import mybir
from concourse.bass_utils import run_bass_kernel_spmd

F32 = mybir.dt.float32
F32R = mybir.dt.float32r
BF16 = mybir.dt.bfloat16
AF = mybir.ActivationFunctionType
ALU = mybir.AluOpType

HID = 128
NT = 128       # tokens per image
NAH = 512      # atoms per core (half of 1024)
NG = 64        # ligand graphs
NI = 4         # images
NCORES = 8

TRACE = False
TRACE_KW = {}
LAST = None

# engine routing for the 64 psum evictions (|y+b| abs pass), tuned by trace.
# Only ACT and DVE can read PSUM (GPSIMD/DMA cannot).
EVICT_PAT = "AD" * 32
# engine for each of the 16 Wq (token-scaled W_int) producer ops (SBUF-only)
WQ_PAT = "P" * 16


_COMPUTE_INSTS = (
    "InstActivation", "InstTensorCopy", "InstTensorScalar", "InstTensorScalarPtr",
    "InstTensorTensor", "InstTensorTensorReduce", "InstTensorReduce", "InstMemSet",
    "InstMatmult", "InstScalarTensorTensor", "InstTensorTensorScan", "InstLdweights",
    "InstDMACopy", "InstDMATransposeAnt", "InstTriggeredCopy", "InstDrain",
    "InstEventSemaphoreOp", "InstSemaphoreOp", "InstCopy", "InstIota", "InstSelect",
)


def _legalize_waits(nc):
    # walrus in this toolchain accepts at most ONE sync wait on TPB compute
    # instructions; hoist extras into same-engine NoOps placed just before.
    k = 0
    for f in nc.m.functions:
        for blk in f.blocks:
            insts = blk.instructions
            out = []
            for ins in insts:
                si = getattr(ins, "sync_info", None)
                if (si is not None and len(si.on_wait) > 1
                        and type(ins).__name__ in _COMPUTE_INSTS):
                    waits = list(si.on_wait)
                    for w in waits[:-1]:
                        nop = mybir.InstNoOp(
                            name=f"WNOP-{k}", engine=ins.engine,
                            sync_info=mybir.SyncInfo(on_wait=[w], on_update=[]))
                        k += 1
                        out.append(nop)
                    ins.sync_info = mybir.SyncInfo(on_wait=[waits[-1]],
                                                   on_update=list(si.on_update))
                out.append(ins)
            blk.instructions = out
    return k


def build_program(bpe: float, bpg: float, bb2: float, bint_zero: bool = True,
                  sim_trace: bool = False) -> bass.Bass:
    nc = bass.Bass()

    # ---- DRAM inputs (per-core views; same names across SPMD cores) ----
    d_bias = nc.dram_tensor("biases", [128, 20], F32, kind="ExternalInput")
    # bf16 pack: wint [0:128], wpeg [128:130], cpe/cpg [130:132]
    d_b16 = nc.dram_tensor("b16pack", [128, 132], BF16, kind="ExternalInput")
    d_tfT = nc.dram_tensor("tfT", [128, 256], F32, kind="ExternalInput")
    d_wtok = nc.dram_tensor("wtok", [128, 256], F32R, kind="ExternalInput")
    d_laT = nc.dram_tensor("laT", [64, NAH], F32R, kind="ExternalInput")
    d_watg = nc.dram_tensor("watg", [64, 256], F32R, kind="ExternalInput")
    d_lgT = nc.dram_tensor("lgT", [64, NG], F32R, kind="ExternalInput")
    d_St = nc.dram_tensor("Sh", [128, 4 * NG], F32, kind="ExternalInput")
    d_msf0 = nc.dram_tensor("msf0", [128, 1024], F32, kind="ExternalInput")
    d_msf1 = nc.dram_tensor("msf1", [64, 512], F32, kind="ExternalInput")
    d_w1s = nc.dram_tensor("W1s", [32, 27 * HID], BF16, kind="ExternalInput")
    d_w0s = nc.dram_tensor("W0s", [64, 27 * HID], BF16, kind="ExternalInput")
    # f32r pack: wpk [0:256], wcat [256:640], wgate [640:1024], wb1 [1024:1280],
    # wb2 [1280:1281]
    d_wbig = nc.dram_tensor("wbig", [128, 1281], F32, kind="ExternalInput")

    d_seg = nc.dram_tensor("seg_out", [1, NG], F32, kind="ExternalOutput")
    d_bias_out = nc.dram_tensor("bias_out", [1, NG], F32, kind="ExternalOutput")

    tc_ref = tile.TileContext(nc, trace_sim=sim_trace)
    with tc_ref as tc:
        with (
            tc.tile_pool(name="const", bufs=1) as cpool,
            tc.tile_pool(name="conv", bufs=1) as vpool,
            tc.tile_pool(name="pre_sb", bufs=2) as prepool,
            tc.tile_pool(name="wq", bufs=4) as wqpool,
            tc.tile_pool(name="h", bufs=4) as hpool,
            tc.tile_pool(name="gate", bufs=2) as gpool,
            tc.tile_pool(name="junk", bufs=2) as jpool,
            tc.tile_pool(name="ps_pre", bufs=1, space="PSUM") as pspre,
            tc.tile_pool(name="ps_y", bufs=3, space="PSUM") as psy,
            tc.tile_pool(name="ps_z", bufs=1, space="PSUM") as psz,
        ):
            # ---------- warm-up: PE pstate ramp + ACT table preload ----------
            warm = cpool.tile([128, 96], BF16, tag="warm")
            nc.gpsimd.memset(warm[:], 0.0)
            wjunk = cpool.tile([128, 16], F32, tag="wjunk")
            nc.scalar.activation(wjunk[:], warm[:, 0:16], AF.Silu)
            ps_warm = pspre.tile([16, 64], F32, tag="ps")
            for wi in range(52):
                nc.tensor.matmul(ps_warm[:], warm[:, 0:16], warm[:, 16:80],
                                 start=(wi == 0), stop=(wi == 51))

            # ---------- critical-path DMAs, spread across SP and Pool queues ----
            la = cpool.tile([64, NAH], F32R, tag="la")
            nc.sync.dma_start(la[:], d_laT[:])
            tf = cpool.tile([128, 256], F32, tag="tf")
            nc.sync.dma_start(tf[:], d_tfT[:])
            wtok = cpool.tile([128, 256], F32R, tag="wtok")
            nc.gpsimd.dma_start(wtok[:], d_wtok[:])
            b16 = cpool.tile([128, 132], BF16, tag="b16")
            nc.gpsimd.dma_start(b16[:], d_b16[:])
            biases = cpool.tile([128, 20], F32, tag="biases")
            nc.gpsimd.dma_start(biases[:], d_bias[:])
            watg = cpool.tile([64, 256], F32R, tag="watg")
            nc.gpsimd.dma_start(watg[:], d_watg[:])
            wint = b16[:, 0:128]
            wpeg = cpool.tile([128, 2], F32R, tag="wpeg")
            nc.scalar.activation(wpeg[:], biases[:, 12:14], AF.Copy)

            btok = biases[:, 0:1]
            bpk = biases[:, 1:2]
            bcat = biases[:, 2:3]
            bgateh = biases[:, 3:4]
            batom = biases[:, 4:5]
            bgraph = biases[:, 5:6]
            bb1 = biases[:, 6:7]
            bint = biases[:, 7:8]
            bc0 = biases[:, 8:9]
            bc1 = biases[:, 9:10]

            # ---------- secondary DMAs (SP queue, overlap with loop) ----------
            msf0 = vpool.tile([128, 1024], F32, tag="msf0")
            nc.sync.dma_start(msf0[:], d_msf0[:])
            w1s = cpool.tile([32, 27 * HID], BF16, tag="w1s")
            nc.sync.dma_start(w1s[:], d_w1s[:])
            w0s = cpool.tile([64, 27 * HID], BF16, tag="w0s")
            nc.sync.dma_start(w0s[:], d_w0s[:])
            wbig = cpool.tile([128, 1281], F32, tag="wbig")
            nc.sync.dma_start(wbig[:], d_wbig[:])
            msf1 = vpool.tile([64, 512], F32, tag="msf1")
            nc.sync.dma_start(msf1[:], d_msf1[:])
            lg = cpool.tile([64, NG], F32R, tag="lg")
            nc.sync.dma_start(lg[:], d_lgT[:])
            St = cpool.tile([128, 4 * NG], F32, tag="St")
            nc.sync.dma_start(St[:], d_St[:])

            # ---------- preamble: tok (tokens 0-15 first to unblock Wq0) ----
            tfr = prepool.tile([128, 256], F32R, tag="tfr")
            ps_tok = pspre.tile([128, NT], F32, tag="ps")
            tokT = cpool.tile([128, NT], F32, tag="tokT")
            for j0, j1 in ((0, 16), (16, 128)):
                nc.scalar.activation(tfr[:, j0:j1], tf[:, j0:j1], AF.Silu)
                nc.scalar.activation(tfr[:, 128 + j0:128 + j1],
                                     tf[:, 128 + j0:128 + j1], AF.Silu)
                nc.tensor.matmul(ps_tok[:, j0:j1], wtok[:, 0:HID], tfr[:, j0:j1],
                                 start=True, stop=False)
                nc.tensor.matmul(ps_tok[:, j0:j1], wtok[:, HID:2 * HID],
                                 tfr[:, 128 + j0:128 + j1], start=False, stop=True)
                nc.scalar.activation(tokT[:, j0:j1], ps_tok[:, j0:j1],
                                     AF.Identity, bias=btok)

            ps_at = psy.tile([128, NAH], F32, tag="y")
            nc.tensor.matmul(ps_at[:], watg[:, 0:128], la[:], start=True, stop=True)
            atomsT = cpool.tile([128, NAH], BF16, tag="atomsT")
            nc.scalar.activation(atomsT[:], ps_at[:], AF.Identity, bias=batom)

            # tokCC: interleaved tokT*c_pe / tokT*c_pg columns (bf16).  Injected
            # into the zq psum accumulation to add the lrelu linear part.
            tokCC = cpool.tile([128, 256], BF16, tag="tokCC")
            nc.gpsimd.tensor_scalar(tokCC[:, 0::2], tokT[:], biases[:, 10:11], 0.0,
                                    op0=ALU.mult, op1=ALU.add)
            nc.gpsimd.tensor_scalar(tokCC[:, 1::2], tokT[:], biases[:, 11:12], 0.0,
                                    op0=ALU.mult, op1=ALU.add)


            # ---------- preamble tasks (interleaved into the main loop) ----------
            state = {}

            def t_conv_silu():
                # silu(msf0) with per-(zr,y)-row leading zero cols for the x-scan
                s0z = vpool.tile([128, 1088], F32, tag="s0z")
                s0v = s0z[:, :].rearrange("p (zr y c) -> p zr y c", zr=4, y=16)
                nc.gpsimd.memset(s0v[:, :, :, 0:1], 0.0)
                m0v = msf0[:, :].rearrange("p (zr y x) -> p zr y x", zr=4, y=16)
                nc.scalar.activation(s0v[:, :, :, 1:17], m0v[:], AF.Silu)
                state["s0z"] = s0z

            def t_conv_xscan():
                P1 = vpool.tile([128, 1088], F32, tag="P1")
                nc.vector.tensor_tensor_scan(P1[:], state["s0z"][:], state["s0z"][:],
                                             0.0, op0=ALU.add, op1=ALU.bypass)
                # A[zr, y, dx] = P[...x=dx+13] - P[...x=dx-1], stored (dx, zr, 17y)
                A1z = vpool.tile([128, 204], F32, tag="A1z")
                a1w = A1z[:, :].rearrange("p (dx zr c) -> p zr c dx", dx=3, zr=4)
                nc.gpsimd.memset(a1w[:, :, 0:1, :], 0.0)
                p1v = P1[:, :].rearrange("p (zr y c) -> p zr y c", zr=4, y=16)
                nc.gpsimd.tensor_sub(a1w[:, :, 1:17, :],
                                     p1v[:, :, :, 14:17], p1v[:, :, :, 0:3])
                state["A1z"] = A1z

            def t_conv_yscan():
                PY1 = vpool.tile([128, 204], F32, tag="PY1")
                nc.vector.tensor_tensor_scan(PY1[:], state["A1z"][:], state["A1z"][:],
                                             0.0, op0=ALU.add, op1=ALU.bypass)
                B1 = vpool.tile([128, 36], F32, tag="B1")
                b1w = B1[:, :].rearrange("p (dx dy zr) -> p dx zr dy", dy=3, zr=4)
                py1 = PY1[:, :].rearrange("p (dx zr c) -> p dx zr c", dx=3, zr=4)
                nc.gpsimd.tensor_sub(b1w[:], py1[:, :, :, 14:17], py1[:, :, :, 0:3])
                state["B1"] = B1

            def t_conv_shuffle():
                # cross-partition fold of the z quarters: [128=(zq,c),36] -> [32, ...]
                Bz1 = vpool.tile([32, 153], F32, tag="Bz1")
                bz1 = Bz1[:, :].rearrange("p (dx dy c) -> p dx dy c", dx=3, dy=3)
                nc.gpsimd.memset(bz1[:, :, :, 0:1], 0.0)
                b1v = state["B1"][:, :].rearrange("p (dx dy zr) -> p dx dy zr", dx=3, dy=3)
                for zq in range(4):
                    nc.sync.dma_start(bz1[:, :, :, 1 + 4 * zq:1 + 4 * zq + 4],
                                      b1v[32 * zq:32 * (zq + 1)])
                state["Bz1"] = Bz1

            def t_conv_zscan():
                PZ1 = vpool.tile([32, 153], F32, tag="PZ1")
                nc.vector.tensor_tensor_scan(PZ1[:], state["Bz1"][:], state["Bz1"][:],
                                             0.0, op0=ALU.add, op1=ALU.bypass)
                m1 = vpool.tile([32, 27], BF16, tag="m1")
                pz1 = PZ1[:, :].rearrange("p (dx dy c) -> p dx dy c", dx=3, dy=3)
                m1v = m1[:, :].rearrange("p (dx dy dz) -> p dx dy dz", dx=3, dy=3)
                nc.gpsimd.tensor_sub(m1v[:], pz1[:, :, :, 14:17], pz1[:, :, :, 0:3])
                state["m1"] = m1

            def t_conv0_pe():
                x0 = vpool.tile([64, 512], BF16, tag="x0")
                nc.scalar.activation(x0[:], msf1[:], AF.Silu)
                x0v = x0[:, :].rearrange("p (z y x) -> p z y x", z=8, y=8)
                ps_c0 = pspre.tile([128, 216], F32, tag="ps")
                out0 = ps_c0[:, :].rearrange("p (a b c) -> p a b c", a=6, b=6)
                for dz in range(3):
                    for dy in range(3):
                        for dx in range(3):
                            ti = dz * 9 + dy * 3 + dx
                            nc.tensor.matmul(out0, w0s[:, ti * HID:(ti + 1) * HID],
                                             x0v[:, dz:dz + 6, dy:dy + 6, dx:dx + 6],
                                             start=(ti == 0), stop=(ti == 26))
                junk0 = jpool.tile([128, 216], F32, tag="junk216")
                p0a = prepool.tile([128, 1], F32, tag="p0a")
                nc.scalar.activation(junk0[:], ps_c0[:], AF.Copy, accum_out=p0a[:])
                sp0 = prepool.tile([128, 1], F32, tag="sp0")
                nc.scalar.activation(sp0[:], p0a[:], AF.Silu, bias=bc0, scale=1.0 / 216.0)
                state["sp0"] = sp0

            def t_pocket():
                # p0 <- conv(ms_feat_1) via W0s @ m0 ; p1 <- conv(ms_feat_0)
                ps_p1 = pspre.tile([128, 1], F32, tag="ps")
                for d in range(27):
                    nc.tensor.matmul(ps_p1[:], w1s[:, d * HID:(d + 1) * HID],
                                     state["m1"][:, d:d + 1],
                                     start=(d == 0), stop=(d == 26))
                sp1 = prepool.tile([128, 1], F32, tag="sp1")
                nc.scalar.activation(sp1[:], ps_p1[:], AF.Silu, bias=bc1)
                sp0 = state["sp0"]
                ps_pk = pspre.tile([128, 1], F32, tag="ps")
                nc.tensor.matmul(ps_pk[:], wbig[:, 0:HID], sp0[:], start=True, stop=False)
                nc.tensor.matmul(ps_pk[:], wbig[:, HID:2 * HID], sp1[:], start=False, stop=True)
                pocket = prepool.tile([128, 1], F32, tag="pocket")
                nc.scalar.activation(pocket[:], ps_pk[:], AF.Identity, bias=bpk)
                state["pocket"] = pocket

            def t_pf():
                tok_sum = prepool.tile([128, 1], F32, tag="toksum")
                junkt = jpool.tile([128, NT], F32, tag="junk")
                nc.vector.tensor_scalar(junkt[:], tokT[:], 1.0, 0.0, op0=ALU.mult,
                                        op1=ALU.add, accum_out=tok_sum[:])
                ps_pf = pspre.tile([128, 2], F32, tag="ps")
                chunks = [state["pocket"], tok_sum, tok_sum]
                for q in range(3):
                    nc.tensor.matmul(ps_pf[:, 0:1], wbig[:, (2 + q) * HID:(3 + q) * HID],
                                     chunks[q][:], start=(q == 0), stop=(q == 2))
                for q in range(3):
                    nc.tensor.matmul(ps_pf[:, 1:2], wbig[:, (5 + q) * HID:(6 + q) * HID],
                                     chunks[q][:], start=(q == 0), stop=(q == 2))
                pf_t = prepool.tile([128, 1], F32, tag="pft")
                nc.scalar.activation(pf_t[:], ps_pf[:, 1:2], AF.Tanh, bias=bgateh, scale=0.5)
                pf_sig = prepool.tile([128, 1], F32, tag="pfsig")
                nc.gpsimd.tensor_scalar(pf_sig[:], pf_t[:], 0.5, 0.5,
                                        op0=ALU.mult, op1=ALU.add)
                pf_lin = prepool.tile([128, 1], F32, tag="pflin")
                nc.scalar.activation(pf_lin[:], ps_pf[:, 0:1], AF.Identity, bias=bcat)
                pf = prepool.tile([128, 1], F32, tag="pf")
                nc.gpsimd.tensor_mul(pf[:], pf_lin[:], pf_sig[:])
                state["pf"] = pf

            def t_bias_head():
                ps_gf = pspre.tile([128, NG], F32, tag="ps")
                nc.tensor.matmul(ps_gf[:], watg[:, 128:256], lg[:], start=True, stop=True)
                gfT = prepool.tile([128, NG], F32, tag="gfT")
                nc.scalar.activation(gfT[:], ps_gf[:], AF.Identity, bias=bgraph)
                ps_u = pspre.tile([128, 1], F32, tag="ps")
                nc.tensor.matmul(ps_u[:], wbig[:, 8 * HID:9 * HID], state["pf"][:],
                                 start=True, stop=True)
                ub = prepool.tile([128, 1], F32, tag="ub")
                nc.scalar.activation(ub[:], ps_u[:], AF.Identity, bias=bb1)
                ps_hb = pspre.tile([128, NG], F32, tag="ps")
                nc.tensor.matmul(ps_hb[:], wbig[:, 9 * HID:10 * HID], gfT[:],
                                 start=True, stop=True)
                hb = prepool.tile([128, NG], F32, tag="hb")
                nc.scalar.activation(hb[:], ps_hb[:], AF.Lrelu, bias=ub[:], alpha=0.01)
                ps_b2 = pspre.tile([1, NG], F32, tag="ps")
                nc.tensor.matmul(ps_b2[:], wbig[:, 10 * HID:10 * HID + 1], hb[:],
                                 start=True, stop=True)
                bias_sb = prepool.tile([1, NG], F32, tag="bias")
                nc.scalar.activation(bias_sb[:], ps_b2[:], AF.Identity, bias=biases[0:1, 16:17])
                nc.sync.dma_start(d_bias_out[:], bias_sb[:])

            pre_tasks = [t_conv_silu, t_conv_xscan, t_conv_yscan, t_conv_shuffle,
                         t_conv_zscan, t_conv0_pe, t_pocket, t_pf, t_bias_head]
            PRE_AT = {(0, 15): [0, 1, 2], (0, 31): [3, 4, 5], (1, 15): [6, 7, 8]}

            # ---------- Wq producers: Wq[j] = wint * tok[:, j] ----------
            ENG = {"A": nc.scalar, "D": nc.vector, "P": nc.gpsimd}
            wint_b = wint[:, None, :].broadcast_to((128, 8, 128))

            wint_b4 = wint[:, None, :].broadcast_to((128, 4, 128))

            def make_wq(b, split=False):
                t = wqpool.tile([128, 1024], BF16, tag="wq")
                tv = t[:, :].rearrange("p (t k) -> p t k", t=8)
                if split:
                    for hh in range(2):
                        tok_b = tokT[:, 8 * b + 4 * hh:8 * b + 4 * hh + 4][:, :, None]                             .broadcast_to((128, 4, 128))
                        ENG[WQ_PAT[b]].tensor_mul(tv[:, 4 * hh:4 * hh + 4, :], wint_b4, tok_b)
                else:
                    tok_b = tokT[:, 8 * b:8 * b + 8][:, :, None].broadcast_to((128, 8, 128))
                    ENG[WQ_PAT[b]].tensor_mul(tv[:], wint_b, tok_b)
                return t

            wq_tiles = {0: make_wq(0, split=True), 1: make_wq(1), 2: make_wq(2), 3: make_wq(3)}

            # ---------- main loop: 2 groups x 64 tokens, 2-token psum tiles ----------
            t_last = None
            for gg in range(2):
                zq = psz.tile([128, 512], F32, tag="z")
                h_prev = None
                jj_prev = 0
                for pair in range(32):
                    gpair = 32 * gg + pair
                    b = gpair // 4
                    if pair % 4 == 0 and b + 4 < 16:
                        wq_tiles[b + 4] = make_wq(b + 4)
                    wq = wq_tiles[b]
                    off = (gpair % 4) * 256
                    # z matmuls of the previous pair first on PE
                    if h_prev is not None:
                        for v in range(2):
                            jj = jj_prev + v
                            j = 64 * gg + jj
                            for c in range(4):
                                col = c * 128 + jj * 2
                                nc.tensor.matmul(zq[:, col:col + 2],
                                                 h_prev[:, v * 512 + c * 128:v * 512 + (c + 1) * 128],
                                                 wpeg, start=True, stop=False)
                                nc.tensor.matmul(zq[:, col:col + 2],
                                                 atomsT[:, c * 128:(c + 1) * 128],
                                                 tokCC[:, 2 * j:2 * j + 2],
                                                 start=False, stop=True)
                    y = psy.tile([128, 1024], F32, tag="y")
                    nc.tensor.matmul(y[:, 0:512], wq[:, off:off + 128], atomsT[:],
                                     start=True, stop=True)
                    nc.tensor.matmul(y[:, 512:1024], wq[:, off + 128:off + 256],
                                     atomsT[:], start=True, stop=True)
                    # relu(y + b_int) eviction; lrelu = 0.99 relu(v) + 0.01 v and
                    # the 0.01 v linear part is injected into zq via tokCC
                    h = hpool.tile([128, 1024], F32R, tag="h")
                    e = EVICT_PAT[gpair]
                    if e == "A":
                        nc.scalar.activation(h[:], y[:], AF.Relu, bias=bint)
                    else:
                        nc.vector.tensor_scalar(h[:], y[:], bint, 0.0,
                                                op0=ALU.add, op1=ALU.max)
                    h_prev = h
                    jj_prev = 2 * pair
                    if (gg, pair) in PRE_AT:
                        for fn in PRE_AT[(gg, pair)]:
                            pre_tasks[fn]()
                for v in range(2):
                    jj = 62 + v
                    j = 64 * gg + jj
                    for c in range(4):
                        col = c * 128 + jj * 2
                        nc.tensor.matmul(zq[:, col:col + 2],
                                         h_prev[:, v * 512 + c * 128:v * 512 + (c + 1) * 128],
                                         wpeg, start=True, stop=False)
                        nc.tensor.matmul(zq[:, col:col + 2],
                                         atomsT[:, c * 128:(c + 1) * 128],
                                         tokCC[:, 2 * j:2 * j + 2],
                                         start=False, stop=True)
                # gating: zq already holds 0.99*relu-dot + linear part
                s = gpool.tile([128, 256], F32, tag="s")
                nc.scalar.activation(s[:], zq[:, 1::2], AF.Tanh, bias=biases[:, 14:15], scale=0.5)
                w = gpool.tile([128, 256], F32, tag="w")
                nc.gpsimd.tensor_scalar(w[:], s[:], 0.5, 0.5, op0=ALU.mult, op1=ALU.add)
                t = gpool.tile([128, 256], F32, tag="t")
                nc.vector.scalar_tensor_tensor(t[:], zq[:, 0::2], biases[:, 15:16], w[:],
                                               op0=ALU.add, op1=ALU.mult)
                if gg == 0:
                    t_last = t
                else:
                    state["t1"] = t

            # ---------- epilogue: atom energies -> seg ----------
            ae4 = prepool.tile([128, 8], F32, tag="ae4")
            t0v = t_last[:, :].rearrange("p (c jj) -> p c jj", c=4)
            t1v = state["t1"][:, :].rearrange("p (c jj) -> p c jj", c=4)
            for c in range(4):
                junka = jpool.tile([128, 64], F32, tag="junk64")
                nc.vector.tensor_scalar(junka[:], t0v[:, c, :], 1.0, 0.0,
                                        op0=ALU.mult, op1=ALU.add,
                                        accum_out=ae4[:, c:c + 1])
            for c in range(4):
                junka = jpool.tile([128, 64], F32, tag="junk64")
                nc.vector.tensor_scalar(junka[:], t1v[:, c, :], 1.0, 0.0,
                                        op0=ALU.mult, op1=ALU.add,
                                        accum_out=ae4[:, 4 + c:5 + c])
            ps_seg = pspre.tile([1, NG], F32, tag="ps")
            for q in range(8):
                nc.tensor.matmul(ps_seg[:], ae4[:, q:q + 1], St[:, (q % 4) * NG:(q % 4 + 1) * NG],
                                 start=(q == 0), stop=(q == 7))
            seg_sb = prepool.tile([1, NG], F32, tag="seg")
            nc.scalar.activation(seg_sb[:], ps_seg[:], AF.Copy)
            nc.sync.dma_start(d_seg[:], seg_sb[:])

    _legalize_waits(nc)
    nc._tile_ctx = tc_ref
    return nc


def kernel(**inputs) -> np.ndarray:
    f = lambda a: np.ascontiguousarray(np.asarray(a), dtype=np.float32)
    tf = f(inputs["token_features"])
    la = f(inputs["lig_atom"])
    lg = f(inputs["lig_graph"])
    m0 = f(inputs["ms_feat_0"])
    m1 = f(inputs["ms_feat_1"])
    lb = np.asarray(inputs["ligand_batch"])
    S = (lb[:, None] == np.arange(NG)[None, :]).astype(np.float32)

    W_int = f(inputs["W_int"])
    w_pe = f(inputs["W_pe"]).reshape(-1)
    w_pg = f(inputs["W_pg"]).reshape(-1)
    b_int = f(inputs["b_int"]).reshape(-1)
    # lrelu(v) = 0.99 relu(v) + 0.01 v; linear part via c = 0.01 * W_int @ w
    c_pe = 0.01 * (W_int @ w_pe)
    c_pg = 0.01 * (W_int @ w_pg)
    bpe_eff = float(np.asarray(inputs["b_pe"]).reshape(-1)[0]
                    + 0.01 * float(w_pe @ b_int))
    bpg_eff = float(np.asarray(inputs["b_pg"]).reshape(-1)[0]
                    + 0.01 * float(w_pg @ b_int))
    bb2 = float(np.asarray(inputs["b_bias2"]).reshape(-1)[0])
    # zq accumulates 0.99*(wpeg . relu(v)) directly plus the linear-part inject
    wpeg = 0.99 * np.concatenate([w_pe[:, None], w_pg[:, None]], axis=1)

    b16pack = np.zeros((128, 132), dtype=np.float32)
    b16pack[:, 0:128] = W_int
    b16pack[:, 128:130] = wpeg
    b16pack[:, 130] = c_pe
    b16pack[:, 131] = c_pg

    biases = np.zeros((128, 20), dtype=np.float32)
    col = lambda a: f(a).reshape(128)
    biases[:, 0] = col(inputs["b_token"])
    biases[:, 1] = col(inputs["b_pocket"])
    biases[:, 2] = col(inputs["b_cat"])
    biases[:, 3] = col(inputs["b_gate"]) * 0.5
    biases[:, 4] = col(inputs["b_atom"])
    biases[:, 5] = col(inputs["b_graph"])
    biases[:, 6] = col(inputs["b_bias1"])
    biases[:, 7] = col(inputs["b_int"])
    biases[:, 8] = col(inputs["bc0"])
    biases[:, 9] = col(inputs["bc1"])
    biases[:, 10] = c_pe
    biases[:, 11] = c_pg
    biases[:, 12:14] = wpeg
    biases[:, 14] = 0.5 * bpg_eff
    biases[:, 15] = bpe_eff
    biases[:, 16] = bb2

    # conv weights: [o, c, dz, dy, dx] -> [c, (dx, dy, dz), o], mean folded in
    Wc1 = f(inputs["Wc1"])
    Wc0 = f(inputs["Wc0"])
    W1s = (np.transpose(Wc1, (1, 4, 3, 2, 0)).reshape(32, 27 * HID)
           / 2744.0).astype(ml_dtypes.bfloat16)
    W0s = (np.transpose(Wc0, (1, 2, 3, 4, 0)).reshape(64, 27 * HID)
           / 216.0).astype(ml_dtypes.bfloat16)

    wcat = f(inputs["W_cat"]).copy()
    wgate = f(inputs["W_gate"]).copy()
    wcat[2 * HID:] /= float(NT)
    wgate[2 * HID:] /= float(NT)
    wbig = np.zeros((128, 1281), dtype=np.float32)
    wbig[:, 0:256] = f(inputs["W_pocket"]).reshape(2, 128, HID).transpose(1, 0, 2).reshape(128, 256)
    wbig[:, 256:640] = wcat.reshape(3, 128, HID).transpose(1, 0, 2).reshape(128, 384)
    wbig[:, 640:1024] = wgate.reshape(3, 128, HID).transpose(1, 0, 2).reshape(128, 384)
    wbig[:, 1024:1280] = f(inputs["W_bias1"]).reshape(2, 128, HID).transpose(1, 0, 2).reshape(128, 256)
    wbig[:, 1280] = f(inputs["W_bias2"]).reshape(-1)

    watg = np.concatenate([f(inputs["W_atom"]), f(inputs["W_graph"])], axis=1)
    # W_token rows chunk q = features [128q, 128q+128), lhsT = [feat, out]
    wtok = np.concatenate([f(inputs["W_token"])[0:128, :],
                           f(inputs["W_token"])[128:256, :]], axis=1)

    shared = {
        "biases": biases,
        "b16pack": b16pack.astype(ml_dtypes.bfloat16),
        "wtok": wtok,
        "watg": watg,
        "W1s": W1s, "W0s": W0s,
        "wbig": wbig,
    }

    in_maps = []
    for cidx in range(NCORES):
        n, h = cidx // 2, cidx % 2
        m = dict(shared)
        tfn = tf[n].T  # [256 feat, 128 tok]
        m["tfT"] = np.concatenate([tfn[0:128], tfn[128:256]], axis=1)
        m["laT"] = np.ascontiguousarray(la[n, 512 * h:512 * (h + 1)].T)
        m["lgT"] = np.ascontiguousarray(lg[n].T)
        # msf0 [32,16,16,16] -> [(zq, c), (zr, y, x)]
        m["msf0"] = np.ascontiguousarray(
            m0[n].reshape(32, 4, 4, 16, 16).transpose(1, 0, 2, 3, 4).reshape(128, 1024))
        m["msf1"] = np.ascontiguousarray(m1[n].reshape(64, 512))
        m["Sh"] = np.ascontiguousarray(
            S[512 * h:512 * (h + 1)].reshape(4, 128, NG).transpose(1, 0, 2).reshape(128, 4 * NG))
        in_maps.append(m)

    nc = build_program(bpe_eff, bpg_eff, bb2)
    r = run_bass_kernel_spmd(nc, in_maps, core_ids=list(range(NCORES)),
                             trace=TRACE, **(TRACE_KW if TRACE else {}))
    global LAST
    LAST = r
    res = r.results

    out = np.zeros((NI, NG), dtype=np.float32)
    for n in range(NI):
        out[n] = (res[2 * n]["seg_out"][0] + res[2 * n + 1]["seg_out"][0]
                  + res[2 * n]["bias_out"][0])
    return out
